# revision 1
# baseline (speedup 1.0000x reference)
"""Deformable PSROI pooling (group_size=1, num_classes=1) on 8 trn2 NeuronCores.

Strategy ("map sweep"):
  out[n, c, ph, pw] = sum_{y,x} KY[bin, y] * KX[bin, x] * data[b, c, y, x]
where KX/KY are per-bin bilinear "hat" weight profiles (sums over the 4x4
sample grid, with sample masks and 1/count folded in).  Each core holds a
slice of one batch's feature map in SBUF in [x(128 partitions), (y, c)]
layout; for each feature row y it issues one TensorE matmul
    psum[c, bins] += map_row[x, c].T @ W_y[x, bins]
accumulating bins in PSUM 512-column "generations" (bins sorted by first
active row).  W_y columns are dense per-bin x-profiles scaled by ky[bin, y],
built on the host and streamed per generation.

Sharding: bins are split by (batch, ylo-quantile) into 8 equal-count shards.
The compiled program is shared by all cores; all per-core variation lives in
the input tensors (map slice, W stream).  Cross-core schedule alignment is
per-generation ("virtual steps"), with the map supplied as per-generation row
segments so each core can anchor a generation at its own starting row.
"""
import sys
import time

import numpy as np

sys.path.insert(0, "/opt/trn_rl_repo")

SPATIAL_SCALE = np.float32(0.0625)
POOLED = 7
SAMPLES = 4
TRANS_STD = np.float32(0.1)
B, C, H, W = 2, 128, 128, 128
NCORES = 8
GEN_COLS = 512
DT_MODE = "f32"  # "f32" (exact, 4-pass PE) or "f32r" (1-pass, ~1.5e-4 rel err)

f32 = np.float32
YSENT = 10 ** 6  # sentinel ylo for bins with all-zero weights


# ----------------------------------------------------------------------------
# host planning
# ----------------------------------------------------------------------------

def _bin_params(rois, offset):
    """Exact float32 emulation of the reference coordinate math.

    Returns per-bin (N*49) arrays: batch, dense hat profiles kx/ky [nb, 128]
    (ky has 1/count folded in), y-support [ylo, yhi], zero-bin mask.
    """
    N = rois.shape[0]
    P, S = POOLED, SAMPLES
    rois = rois.astype(f32)
    offset = offset.astype(f32)

    batch_ind = rois[:, 0].astype(np.int32)
    roi_sw = np.round(rois[:, 1]) * SPATIAL_SCALE - f32(0.5)
    roi_sh = np.round(rois[:, 2]) * SPATIAL_SCALE - f32(0.5)
    roi_ew = np.round(rois[:, 3] + f32(1.0)) * SPATIAL_SCALE - f32(0.5)
    roi_eh = np.round(rois[:, 4] + f32(1.0)) * SPATIAL_SCALE - f32(0.5)
    roi_w = np.maximum(roi_ew - roi_sw, f32(0.1))
    roi_h = np.maximum(roi_eh - roi_sh, f32(0.1))
    bin_w = roi_w / f32(P)
    bin_h = roi_h / f32(P)
    sub_w = bin_w / f32(S)
    sub_h = bin_h / f32(S)

    pidx = np.arange(P, dtype=f32)
    trans_x = offset[:, 0] * TRANS_STD          # [N, 7(ph), 7(pw)]
    trans_y = offset[:, 1] * TRANS_STD
    pw = pidx[None, None, :]
    ph = pidx[None, :, None]
    wstart = pw * bin_w[:, None, None] + roi_sw[:, None, None] + trans_x * roi_w[:, None, None]
    hstart = ph * bin_h[:, None, None] + roi_sh[:, None, None] + trans_y * roi_h[:, None, None]

    sidx = np.arange(S, dtype=f32)
    w_s = wstart[..., None] + sidx * sub_w[:, None, None, None]     # [N,7,7,4]
    h_s = hstart[..., None] + sidx * sub_h[:, None, None, None]
    mask_w = (w_s >= f32(-0.5)) & (w_s <= f32(W) - f32(0.5))
    mask_h = (h_s >= f32(-0.5)) & (h_s <= f32(H) - f32(0.5))
    wc = np.clip(w_s, f32(0.0), f32(W - 1))
    hc = np.clip(h_s, f32(0.0), f32(H - 1))

    cnt = (mask_h.sum(-1) * mask_w.sum(-1)).astype(f32)             # [N,7,7]
    inv = np.where(cnt > 0, f32(1.0) / np.maximum(cnt, f32(1.0)), f32(0.0))

    nb = N * P * P
    wc = wc.reshape(nb, S)
    hc = hc.reshape(nb, S)
    mask_w = mask_w.reshape(nb, S)
    mask_h = mask_h.reshape(nb, S)
    inv = inv.reshape(nb)

    xg = np.arange(W, dtype=np.float64)
    kx = np.zeros((nb, W), np.float64)
    ky = np.zeros((nb, H), np.float64)
    for s in range(S):
        kx += mask_w[:, s, None] * np.maximum(0.0, 1.0 - np.abs(wc[:, s, None].astype(np.float64) - xg))
        ky += mask_h[:, s, None] * np.maximum(0.0, 1.0 - np.abs(hc[:, s, None].astype(np.float64) - xg))
    ky *= inv[:, None]
    kx = kx.astype(f32)
    ky = ky.astype(f32)

    ky_nz = ky != 0
    has_y = ky_nz.any(axis=1)
    ylo = np.where(has_y, ky_nz.argmax(axis=1), YSENT).astype(np.int64)
    yhi = np.where(has_y, H - 1 - ky_nz[:, ::-1].argmax(axis=1), -YSENT).astype(np.int64)

    batch = np.repeat(batch_ind, P * P)
    return batch, kx, ky, ylo, yhi, has_y


def _plan(rois, offset):
    batch, kx, ky, ylo, yhi, has_y = _bin_params(rois, offset)

    # shard bins: (batch, ylo-quantile) -> 8 shards with equal-ish counts
    shards = []
    for b in range(B):
        ids = np.where(batch == b)[0]
        # secondary yhi sort shrinks the retirement-prefix blocking window
        ids = ids[np.lexsort((yhi[ids], ylo[ids]))]
        q = NCORES // B
        shards.extend(ids[int(len(ids) * i / q):int(len(ids) * (i + 1) / q)]
                      for i in range(q))
    assert len(shards) == NCORES

    max_count = max(len(s) for s in shards)
    ngens = max(1, -(-max_count // GEN_COLS))
    nslots = ngens * GEN_COLS

    # per-shard absolute row window
    row_start = np.zeros(NCORES, np.int64)
    extents = []
    for ids in shards:
        real = ids[ylo[ids] < YSENT]
        if len(real):
            extents.append((int(ylo[real].min()), int(yhi[real].max())))
        else:
            extents.append((0, 0))
    rstar = max(b_ - a_ + 1 for a_, b_ in extents)
    rstar = min(H, -(-rstar // 8) * 8)
    for ci, (a_, b_) in enumerate(extents):
        row_start[ci] = min(a_, H - rstar)

    # per (core, gen): local anchor t0 and per-step lo/hi profiles
    t0 = np.zeros((NCORES, ngens), np.int64)         # anchor row (local)
    span = np.zeros((NCORES, ngens), np.int64)       # steps used by this core
    profiles = {}                                    # (ci, g) -> (los, his) arrays
    for ci, ids in enumerate(shards):
        rs = int(row_start[ci])
        for g in range(ngens):
            gids = ids[g * GEN_COLS:(g + 1) * GEN_COLS]
            real_mask = ylo[gids] < YSENT
            real = gids[real_mask]
            if len(real) == 0:
                continue
            yl_r = ylo[real] - rs
            yh_r = yhi[real] - rs
            a_, b_ = int(yl_r.min()), int(yh_r.max())
            t0[ci, g] = a_
            span[ci, g] = b_ - a_ + 1
            # profiles over the gen's slot list (real bins are a prefix of the
            # slot list because sentinels sort last)
            yl = ylo[gids] - rs
            yh = yhi[gids] - rs
            los, his = [], []
            for s in range(b_ - a_ + 1):
                t = a_ + s
                his.append(int(np.count_nonzero(yl <= t)))
                live = np.where(yh >= t)[0]
                los.append(int(live[0]) if len(live) else his[-1])
            profiles[(ci, g)] = (los, his)

    # shared schedule: per gen, steps s in [0, S_g); envelope widths
    sched = []          # (g, s, col_lo, col_hi, first, last)
    seg_rows = []       # steps per gen (map segment sizes)
    for g in range(ngens):
        S_g = int(span[:, g].max()) if span[:, g].max() > 0 else 0
        seg_rows.append(S_g)
        if S_g == 0:
            continue
        al = 8 if DT_MODE == "f32r" else 1   # fp32r ISA needs 8-aligned psum cols
        rows = []
        for s in range(S_g):
            cl, ch = YSENT, 0
            for ci in range(NCORES):
                if (ci, g) not in profiles:
                    continue
                los, his = profiles[(ci, g)]
                if s < len(los) and his[s] > los[s]:
                    cl = min(cl, los[s])
                    ch = max(ch, his[s])
            if ch > cl:
                rows.append((s, cl & ~(al - 1), min(GEN_COLS, -(-ch // al) * al)))
        for i, (s, cl, ch) in enumerate(rows):
            sched.append((g, s, cl, ch, i == 0, i == len(rows) - 1))
    wcols = sum(ch - cl for (_, _, cl, ch, _, _) in sched)
    srows = sum(seg_rows)

    # map segment offsets (rows) and W chunk offsets (cols) per gen
    seg_off = np.concatenate([[0], np.cumsum(seg_rows)]).astype(np.int64)
    gen_wc = {}
    off = 0
    for (g, s, cl, ch, first, last) in sched:
        if first:
            gen_wc[g] = off
        off += ch - cl

    meta = dict(rstar=int(rstar), ngens=int(ngens), nslots=int(nslots),
                sched=tuple(sched), wcols=int(wcols), srows=int(srows),
                seg_rows=tuple(seg_rows), seg_off=tuple(int(x) for x in seg_off),
                gen_wc=tuple(sorted(gen_wc.items())))
    return dict(meta=meta, shards=shards, row_start=row_start, t0=t0,
                kx=kx, ky=ky, ylo=ylo, has_y=has_y)


def _build_inputs(plan, data):
    meta = plan["meta"]
    sched, srows, wcols = meta["sched"], meta["srows"], meta["wcols"]
    seg_off = meta["seg_off"]
    kx, ky = plan["kx"], plan["ky"]
    t0 = plan["t0"]
    data_perm = np.ascontiguousarray(data.transpose(0, 3, 2, 1))  # [B, W(x), H(y), C]

    in_maps = []
    for ci in range(NCORES):
        ids = plan["shards"][ci]
        b = ci // (NCORES // B)
        rs = int(plan["row_start"][ci])
        # segmented map: for gen g, S_g rows starting at rs + t0[ci, g]
        mp = np.zeros((128, srows, C), f32)
        for g, S_g in enumerate(meta["seg_rows"]):
            if S_g == 0:
                continue
            y0 = rs + int(t0[ci, g])
            y1 = min(H, y0 + S_g)
            mp[:, seg_off[g]:seg_off[g] + (y1 - y0), :] = data_perm[b, :, y0:y1, :]
        wbuf = np.zeros((128, wcols), f32)
        wc_off = 0
        for (g, s, cl, ch, first, last) in sched:
            width = ch - cl
            y = rs + int(t0[ci, g]) + s
            if y < H:
                gids = ids[g * GEN_COLS + cl:g * GEN_COLS + ch]
                if len(gids):
                    vals = kx[gids] * ky[gids, y][:, None]      # [ncols_real, 128]
                    wbuf[:, wc_off:wc_off + len(gids)] = vals.T
            wc_off += width
        in_maps.append({"mp": np.ascontiguousarray(mp.reshape(128, srows * C)),
                        "w": wbuf})
    return in_maps


# ----------------------------------------------------------------------------
# device program
# ----------------------------------------------------------------------------

def _split_drains(nc, mybir, bass_rust):
    for f_ in nc.m.functions:
        for blk in f_.blocks:
            newlist = []
            for ins in blk.instructions:
                wts = list(ins.sync_info.on_wait) if ins.sync_info else []
                if len(wts) > 1 and type(ins).__name__ == "InstDrain":
                    for j, wx in enumerate(wts[1:]):
                        nop = mybir.InstNoOp(name=f"splitw_{id(ins)}_{j}", ins=[], outs=[])
                        nop.engine = ins.engine
                        nop.sync_info = bass_rust.SyncInfo(on_wait=[wx], on_update=[])
                        newlist.append(nop)
                    ins.sync_info.on_wait = wts[:1]
                newlist.append(ins)
            blk.instructions = newlist


def _build_program(meta, rep=1):
    import concourse.bacc as bacc
    import concourse.mybir as mybir
    import bass_rust
    from concourse.tile import TileContext

    ngens, nslots = meta["ngens"], meta["nslots"]
    sched, wcols, srows = meta["sched"], meta["wcols"], meta["srows"]
    seg_off = meta["seg_off"]
    gen_wc = dict(meta["gen_wc"])
    dt = mybir.dt.float32r if DT_MODE == "f32r" else mybir.dt.float32

    # per-gen W chunk extents
    gen_wend = {}
    off = 0
    for (g, s, cl, ch, first, last) in sched:
        off += ch - cl
        gen_wend[g] = off

    nc = bacc.Bacc()
    mp = nc.declare_dram_parameter("mp", [128, srows * C], dt, isOutput=False)
    w = nc.declare_dram_parameter("w", [128, max(wcols, 8)], dt, isOutput=False)
    o = nc.declare_dram_parameter("o", [128, nslots], mybir.dt.float32, isOutput=True)

    with TileContext(nc) as tc:
        with (
            tc.tile_pool(name="const", bufs=1) as constp,
            tc.tile_pool(name="mapp", bufs=2) as mpool,
            tc.tile_pool(name="wp", bufs=3) as wpool,
            tc.tile_pool(name="ps", bufs=2, space="PSUM") as pspool,
        ):
            stage = constp.tile([128, nslots], mybir.dt.float32)
            wmax = max((gen_wend[g] - gen_wc[g] for g in gen_wc), default=8)
            for _rep in range(rep):
                map_t = mpool.tile([128, srows * C], dt, tag="map")
                nload = min(16, srows)
                bounds = [int(srows * i / nload) for i in range(nload + 1)]
                for i in range(nload):
                    r0, r1 = bounds[i], bounds[i + 1]
                    if r1 > r0:
                        nc.sync.dma_start(out=map_t[:, r0 * C:r1 * C], in_=mp[:, r0 * C:r1 * C])
                ps = None
                w_t = None
                cur_g = -1
                wc_off = 0
                for (g, s, cl, ch, first, last) in sched:
                    width = ch - cl
                    if g != cur_g:
                        ps = pspool.tile([128, GEN_COLS], mybir.dt.float32, tag="ps")
                        w_t = wpool.tile([128, wmax], dt, tag="wt")
                        # W stream on the ACT HWDGE ring so it doesn't queue
                        # behind map-segment loads on the SP ring
                        nc.scalar.dma_start(out=w_t[:, :gen_wend[g] - gen_wc[g]],
                                            in_=w[:, gen_wc[g]:gen_wend[g]])
                        cur_g = g
                        wc_off = 0
                    row = map_t[:, (seg_off[g] + s) * C:(seg_off[g] + s + 1) * C]
                    nc.tensor.matmul(ps[:, cl:ch], row, w_t[:, wc_off:wc_off + width],
                                     start=first, stop=last)
                    wc_off += width
                    if last:
                        sl = slice(g * GEN_COLS, (g + 1) * GEN_COLS)
                        nc.vector.tensor_copy(stage[:, sl], ps[:])
                        # drain each generation's output immediately so the
                        # store overlaps later generations' compute
                        nc.sync.dma_start(out=o[:, sl], in_=stage[:, sl])

    _split_drains(nc, mybir, bass_rust)
    nc.finalize()
    return nc


_prog_cache = {}


def _get_program(meta, rep=1):
    key = (meta["sched"], meta["srows"], meta["nslots"], rep, DT_MODE)
    if key not in _prog_cache:
        _prog_cache[key] = _build_program(meta, rep=rep)
    return _prog_cache[key]


def _run(nc, in_maps):
    from concourse.bass_utils import run_bass_kernel_spmd
    last_err = None
    for _attempt in range(3):
        try:
            res = run_bass_kernel_spmd(nc, in_maps, list(range(NCORES)))
            return res.results
        except Exception as e:  # transient device wedge -> retry
            last_err = e
            time.sleep(2.0)
    raise last_err


# ----------------------------------------------------------------------------
# public entry
# ----------------------------------------------------------------------------

def kernel(data, rois, offset):
    data = np.asarray(data, f32)
    rois = np.asarray(rois, f32)
    offset = np.asarray(offset, f32)
    N = rois.shape[0]

    plan = _plan(rois, offset)
    if len(plan["meta"]["sched"]) == 0:   # every bin fully masked
        return np.zeros((N, C, POOLED, POOLED), f32)
    in_maps = _build_inputs(plan, data)
    nc = _get_program(plan["meta"])
    results = _run(nc, in_maps)

    flat = np.zeros((N * POOLED * POOLED, C), f32)   # [bin, c]
    for ci in range(NCORES):
        ids = plan["shards"][ci]
        if len(ids) == 0:
            continue
        sb = results[ci]["o"]  # [128, nslots]
        flat[ids] = sb[:, :len(ids)].T
    flat[~plan["has_y"]] = 0.0   # degenerate bins never touched on device
    out = flat.reshape(N, POOLED, POOLED, C).transpose(0, 3, 1, 2)
    return np.ascontiguousarray(out)



# revision 6
# speedup vs baseline: 1.5202x; 1.5202x over previous
"""Deformable PSROI pooling (group_size=1, num_classes=1) on 8 trn2 NeuronCores.

Strategy ("map sweep"):
  out[n, c, ph, pw] = sum_{y,x} KY[bin, y] * KX[bin, x] * data[b, c, y, x]
where KX/KY are per-bin bilinear "hat" weight profiles (sums over the 4x4
sample grid, with sample masks and 1/count folded in).  Each core holds a
slice of one batch's feature map in SBUF in [x(128 partitions), (y, c)]
layout; for each feature row y it issues one TensorE matmul
    psum[c, bins] += map_row[x, c].T @ W_y[x, bins]
accumulating bins in PSUM 512-column "generations" (bins sorted by first
active row).  W_y columns are dense per-bin x-profiles scaled by ky[bin, y],
built on the host and streamed per generation.

Sharding: bins are split by (batch, ylo-quantile) into 8 equal-count shards.
The compiled program is shared by all cores; all per-core variation lives in
the input tensors (map slice, W stream).  Cross-core schedule alignment is
per-generation ("virtual steps"), with the map supplied as per-generation row
segments so each core can anchor a generation at its own starting row.
"""
import sys
import time

import numpy as np

sys.path.insert(0, "/opt/trn_rl_repo")

SPATIAL_SCALE = np.float32(0.0625)
POOLED = 7
SAMPLES = 4
TRANS_STD = np.float32(0.1)
B, C, H, W = 2, 128, 128, 128
NCORES = 8
GEN_COLS = 512
DT_MODE = "bf16"  # "f32" (exact, 4-pass PE), "f32r" (1-pass), "bf16" (1-pass, half DMA)

f32 = np.float32


def _np_stream_dt():
    if DT_MODE == "bf16":
        import ml_dtypes
        return ml_dtypes.bfloat16
    return f32
YSENT = 10 ** 6  # sentinel ylo for bins with all-zero weights


# ----------------------------------------------------------------------------
# host planning
# ----------------------------------------------------------------------------

def _bin_params(rois, offset):
    """Exact float32 emulation of the reference coordinate math.

    Returns per-bin (N*49) arrays: batch, dense hat profiles kx/ky [nb, 128]
    (ky has 1/count folded in), y-support [ylo, yhi], zero-bin mask.
    """
    N = rois.shape[0]
    P, S = POOLED, SAMPLES
    rois = rois.astype(f32)
    offset = offset.astype(f32)

    batch_ind = rois[:, 0].astype(np.int32)
    roi_sw = np.round(rois[:, 1]) * SPATIAL_SCALE - f32(0.5)
    roi_sh = np.round(rois[:, 2]) * SPATIAL_SCALE - f32(0.5)
    roi_ew = np.round(rois[:, 3] + f32(1.0)) * SPATIAL_SCALE - f32(0.5)
    roi_eh = np.round(rois[:, 4] + f32(1.0)) * SPATIAL_SCALE - f32(0.5)
    roi_w = np.maximum(roi_ew - roi_sw, f32(0.1))
    roi_h = np.maximum(roi_eh - roi_sh, f32(0.1))
    bin_w = roi_w / f32(P)
    bin_h = roi_h / f32(P)
    sub_w = bin_w / f32(S)
    sub_h = bin_h / f32(S)

    pidx = np.arange(P, dtype=f32)
    trans_x = offset[:, 0] * TRANS_STD          # [N, 7(ph), 7(pw)]
    trans_y = offset[:, 1] * TRANS_STD
    pw = pidx[None, None, :]
    ph = pidx[None, :, None]
    wstart = pw * bin_w[:, None, None] + roi_sw[:, None, None] + trans_x * roi_w[:, None, None]
    hstart = ph * bin_h[:, None, None] + roi_sh[:, None, None] + trans_y * roi_h[:, None, None]

    sidx = np.arange(S, dtype=f32)
    w_s = wstart[..., None] + sidx * sub_w[:, None, None, None]     # [N,7,7,4]
    h_s = hstart[..., None] + sidx * sub_h[:, None, None, None]
    mask_w = (w_s >= f32(-0.5)) & (w_s <= f32(W) - f32(0.5))
    mask_h = (h_s >= f32(-0.5)) & (h_s <= f32(H) - f32(0.5))
    wc = np.clip(w_s, f32(0.0), f32(W - 1))
    hc = np.clip(h_s, f32(0.0), f32(H - 1))

    cnt = (mask_h.sum(-1) * mask_w.sum(-1)).astype(f32)             # [N,7,7]
    inv = np.where(cnt > 0, f32(1.0) / np.maximum(cnt, f32(1.0)), f32(0.0))

    nb = N * P * P
    wc = wc.reshape(nb, S)
    hc = hc.reshape(nb, S)
    mask_w = mask_w.reshape(nb, S)
    mask_h = mask_h.reshape(nb, S)
    inv = inv.reshape(nb)

    xg = np.arange(W, dtype=np.float64)
    kx = np.zeros((nb, W), np.float64)
    ky = np.zeros((nb, H), np.float64)
    for s in range(S):
        kx += mask_w[:, s, None] * np.maximum(0.0, 1.0 - np.abs(wc[:, s, None].astype(np.float64) - xg))
        ky += mask_h[:, s, None] * np.maximum(0.0, 1.0 - np.abs(hc[:, s, None].astype(np.float64) - xg))
    ky *= inv[:, None]
    kx = kx.astype(f32)
    ky = ky.astype(f32)

    ky_nz = ky != 0
    has_y = ky_nz.any(axis=1)
    ylo = np.where(has_y, ky_nz.argmax(axis=1), YSENT).astype(np.int64)
    yhi = np.where(has_y, H - 1 - ky_nz[:, ::-1].argmax(axis=1), -YSENT).astype(np.int64)

    batch = np.repeat(batch_ind, P * P)
    return batch, kx, ky, ylo, yhi, has_y


def _plan(rois, offset):
    batch, kx, ky, ylo, yhi, has_y = _bin_params(rois, offset)

    # shard bins: (batch, ylo-quantile) -> 8 shards with equal-ish counts
    shards = []
    for b in range(B):
        ids = np.where(batch == b)[0]
        # secondary yhi sort shrinks the retirement-prefix blocking window
        ids = ids[np.lexsort((yhi[ids], ylo[ids]))]
        q = NCORES // B
        shards.extend(ids[int(len(ids) * i / q):int(len(ids) * (i + 1) / q)]
                      for i in range(q))
    assert len(shards) == NCORES

    max_count = max(len(s) for s in shards)
    ngens = max(1, -(-max_count // GEN_COLS))
    nslots = ngens * GEN_COLS

    # per-shard absolute row window
    row_start = np.zeros(NCORES, np.int64)
    extents = []
    for ids in shards:
        real = ids[ylo[ids] < YSENT]
        if len(real):
            extents.append((int(ylo[real].min()), int(yhi[real].max())))
        else:
            extents.append((0, 0))
    rstar = max(b_ - a_ + 1 for a_, b_ in extents)
    rstar = min(H, -(-rstar // 8) * 8)
    for ci, (a_, b_) in enumerate(extents):
        row_start[ci] = min(a_, H - rstar)

    # per (core, gen): local anchor t0 and per-step lo/hi profiles
    t0 = np.zeros((NCORES, ngens), np.int64)         # anchor row (local)
    span = np.zeros((NCORES, ngens), np.int64)       # steps used by this core
    profiles = {}                                    # (ci, g) -> (los, his) arrays
    for ci, ids in enumerate(shards):
        rs = int(row_start[ci])
        for g in range(ngens):
            gids = ids[g * GEN_COLS:(g + 1) * GEN_COLS]
            real_mask = ylo[gids] < YSENT
            real = gids[real_mask]
            if len(real) == 0:
                continue
            yl_r = ylo[real] - rs
            yh_r = yhi[real] - rs
            a_, b_ = int(yl_r.min()), int(yh_r.max())
            t0[ci, g] = a_
            span[ci, g] = b_ - a_ + 1
            # profiles over the gen's slot list (real bins are a prefix of the
            # slot list because sentinels sort last)
            yl = ylo[gids] - rs
            yh = yhi[gids] - rs
            los, his = [], []
            for s in range(b_ - a_ + 1):
                t = a_ + s
                his.append(int(np.count_nonzero(yl <= t)))
                live = np.where(yh >= t)[0]
                los.append(int(live[0]) if len(live) else his[-1])
            profiles[(ci, g)] = (los, his)

    # shared schedule: per gen, steps s in [0, S_g); envelope widths
    sched = []          # (g, s, col_lo, col_hi, first, last)
    seg_rows = []       # steps per gen (map segment sizes)
    for g in range(ngens):
        S_g = int(span[:, g].max()) if span[:, g].max() > 0 else 0
        seg_rows.append(S_g)
        if S_g == 0:
            continue
        al = 8 if DT_MODE == "f32r" else 1   # fp32r ISA needs 8-aligned psum cols
        rows = []
        for s in range(S_g):
            cl, ch = YSENT, 0
            for ci in range(NCORES):
                if (ci, g) not in profiles:
                    continue
                los, his = profiles[(ci, g)]
                if s < len(los) and his[s] > los[s]:
                    cl = min(cl, los[s])
                    ch = max(ch, his[s])
            if ch > cl:
                rows.append((s, cl & ~(al - 1), min(GEN_COLS, -(-ch // al) * al)))
        for i, (s, cl, ch) in enumerate(rows):
            sched.append((g, s, cl, ch, i == 0, i == len(rows) - 1))
    wcols = sum(ch - cl for (_, _, cl, ch, _, _) in sched)
    srows = sum(seg_rows)

    # map segment offsets (rows) and W chunk offsets (cols) per gen
    seg_off = np.concatenate([[0], np.cumsum(seg_rows)]).astype(np.int64)
    gen_wc = {}
    off = 0
    for (g, s, cl, ch, first, last) in sched:
        if first:
            gen_wc[g] = off
        off += ch - cl

    meta = dict(rstar=int(rstar), ngens=int(ngens), nslots=int(nslots),
                sched=tuple(sched), wcols=int(wcols), srows=int(srows),
                seg_rows=tuple(seg_rows), seg_off=tuple(int(x) for x in seg_off),
                gen_wc=tuple(sorted(gen_wc.items())))
    return dict(meta=meta, shards=shards, row_start=row_start, t0=t0,
                kx=kx, ky=ky, ylo=ylo, has_y=has_y)


def _build_inputs(plan, data):
    meta = plan["meta"]
    sched, srows, wcols = meta["sched"], meta["srows"], meta["wcols"]
    seg_off = meta["seg_off"]
    kx, ky = plan["kx"], plan["ky"]
    t0 = plan["t0"]
    sdt = _np_stream_dt()
    data_perm = np.ascontiguousarray(data.transpose(0, 3, 2, 1)).astype(sdt)  # [B, W(x), H(y), C]

    in_maps = []
    for ci in range(NCORES):
        ids = plan["shards"][ci]
        b = ci // (NCORES // B)
        rs = int(plan["row_start"][ci])
        # segmented map: for gen g, S_g rows starting at rs + t0[ci, g]
        mp = np.zeros((128, srows, C), sdt)
        for g, S_g in enumerate(meta["seg_rows"]):
            if S_g == 0:
                continue
            y0 = rs + int(t0[ci, g])
            y1 = min(H, y0 + S_g)
            mp[:, seg_off[g]:seg_off[g] + (y1 - y0), :] = data_perm[b, :, y0:y1, :]
        wbuf = np.zeros((128, wcols), sdt)
        wc_off = 0
        for (g, s, cl, ch, first, last) in sched:
            width = ch - cl
            y = rs + int(t0[ci, g]) + s
            if y < H:
                gids = ids[g * GEN_COLS + cl:g * GEN_COLS + ch]
                if len(gids):
                    vals = kx[gids] * ky[gids, y][:, None]      # [ncols_real, 128]
                    wbuf[:, wc_off:wc_off + len(gids)] = vals.T.astype(sdt)
            wc_off += width
        in_maps.append({"mp": np.ascontiguousarray(mp.reshape(128, srows * C)),
                        "w": wbuf})
    return in_maps


# ----------------------------------------------------------------------------
# device program
# ----------------------------------------------------------------------------

def _split_drains(nc, mybir, bass_rust):
    for f_ in nc.m.functions:
        for blk in f_.blocks:
            newlist = []
            for ins in blk.instructions:
                wts = list(ins.sync_info.on_wait) if ins.sync_info else []
                if len(wts) > 1 and type(ins).__name__ == "InstDrain":
                    for j, wx in enumerate(wts[1:]):
                        nop = mybir.InstNoOp(name=f"splitw_{id(ins)}_{j}", ins=[], outs=[])
                        nop.engine = ins.engine
                        nop.sync_info = bass_rust.SyncInfo(on_wait=[wx], on_update=[])
                        newlist.append(nop)
                    ins.sync_info.on_wait = wts[:1]
                newlist.append(ins)
            blk.instructions = newlist


def _build_program(meta, rep=1):
    import concourse.bacc as bacc
    import concourse.mybir as mybir
    import bass_rust
    from concourse.tile import TileContext

    ngens, nslots = meta["ngens"], meta["nslots"]
    sched, wcols, srows = meta["sched"], meta["wcols"], meta["srows"]
    seg_off = meta["seg_off"]
    gen_wc = dict(meta["gen_wc"])
    dt = {"f32r": mybir.dt.float32r, "bf16": mybir.dt.bfloat16}.get(DT_MODE, mybir.dt.float32)
    odt = mybir.dt.bfloat16 if DT_MODE == "bf16" else mybir.dt.float32

    # per-gen W chunk extents
    gen_wend = {}
    off = 0
    for (g, s, cl, ch, first, last) in sched:
        off += ch - cl
        gen_wend[g] = off

    nc = bacc.Bacc()
    mp = nc.declare_dram_parameter("mp", [128, srows * C], dt, isOutput=False)
    w = nc.declare_dram_parameter("w", [128, max(wcols, 8)], dt, isOutput=False)
    o = nc.declare_dram_parameter("o", [128, nslots], odt, isOutput=True)

    with TileContext(nc) as tc:
        with (
            tc.tile_pool(name="const", bufs=1) as constp,
            tc.tile_pool(name="mapp", bufs=2) as mpool,
            tc.tile_pool(name="wp", bufs=3) as wpool,
            tc.tile_pool(name="ps", bufs=2, space="PSUM") as pspool,
        ):
            stage = constp.tile([128, nslots], odt)
            wmax = max((gen_wend[g] - gen_wc[g] for g in gen_wc), default=8)
            for _rep in range(rep):
                map_t = mpool.tile([128, srows * C], dt, tag="map")
                nload = min(16, srows)
                bounds = [int(srows * i / nload) for i in range(nload + 1)]
                for i in range(nload):
                    r0, r1 = bounds[i], bounds[i + 1]
                    if r1 > r0:
                        nc.sync.dma_start(out=map_t[:, r0 * C:r1 * C], in_=mp[:, r0 * C:r1 * C])
                ps = None
                w_t = None
                cur_g = -1
                wc_off = 0
                for (g, s, cl, ch, first, last) in sched:
                    width = ch - cl
                    if g != cur_g:
                        ps = pspool.tile([128, GEN_COLS], mybir.dt.float32, tag="ps")
                        w_t = wpool.tile([128, wmax], dt, tag="wt")
                        # W stream on the ACT HWDGE ring so it doesn't queue
                        # behind map-segment loads on the SP ring
                        nc.scalar.dma_start(out=w_t[:, :gen_wend[g] - gen_wc[g]],
                                            in_=w[:, gen_wc[g]:gen_wend[g]])
                        cur_g = g
                        wc_off = 0
                    row = map_t[:, (seg_off[g] + s) * C:(seg_off[g] + s + 1) * C]
                    nc.tensor.matmul(ps[:, cl:ch], row, w_t[:, wc_off:wc_off + width],
                                     start=first, stop=last)
                    wc_off += width
                    if last:
                        sl = slice(g * GEN_COLS, (g + 1) * GEN_COLS)
                        nc.vector.tensor_copy(stage[:, sl], ps[:])
                        # drain each generation's output immediately so the
                        # store overlaps later generations' compute
                        nc.sync.dma_start(out=o[:, sl], in_=stage[:, sl])

    _split_drains(nc, mybir, bass_rust)
    nc.finalize()
    return nc


_prog_cache = {}


def _get_program(meta, rep=1):
    key = (meta["sched"], meta["srows"], meta["nslots"], rep, DT_MODE)
    if key not in _prog_cache:
        _prog_cache[key] = _build_program(meta, rep=rep)
    return _prog_cache[key]


def _run(nc, in_maps):
    from concourse.bass_utils import run_bass_kernel_spmd
    last_err = None
    for _attempt in range(3):
        try:
            res = run_bass_kernel_spmd(nc, in_maps, list(range(NCORES)))
            return res.results
        except Exception as e:  # transient device wedge -> retry
            last_err = e
            time.sleep(2.0)
    raise last_err


# ----------------------------------------------------------------------------
# public entry
# ----------------------------------------------------------------------------

def kernel(data, rois, offset):
    data = np.asarray(data, f32)
    rois = np.asarray(rois, f32)
    offset = np.asarray(offset, f32)
    N = rois.shape[0]

    plan = _plan(rois, offset)
    if len(plan["meta"]["sched"]) == 0:   # every bin fully masked
        return np.zeros((N, C, POOLED, POOLED), f32)
    in_maps = _build_inputs(plan, data)
    nc = _get_program(plan["meta"])
    results = _run(nc, in_maps)

    flat = np.zeros((N * POOLED * POOLED, C), f32)   # [bin, c]
    for ci in range(NCORES):
        ids = plan["shards"][ci]
        if len(ids) == 0:
            continue
        sb = np.asarray(results[ci]["o"]).astype(f32)  # [128, nslots]
        flat[ids] = sb[:, :len(ids)].T
    flat[~plan["has_y"]] = 0.0   # degenerate bins never touched on device
    out = flat.reshape(N, POOLED, POOLED, C).transpose(0, 3, 1, 2)
    return np.ascontiguousarray(out)



# revision 10
# speedup vs baseline: 1.5488x; 1.0188x over previous
"""Deformable PSROI pooling (group_size=1, num_classes=1) on 8 trn2 NeuronCores.

Strategy ("map sweep"):
  out[n, c, ph, pw] = sum_{y,x} KY[bin, y] * KX[bin, x] * data[b, c, y, x]
where KX/KY are per-bin bilinear "hat" weight profiles (sums over the 4x4
sample grid, with sample masks and 1/count folded in).  Each core holds a
slice of one batch's feature map in SBUF in [x(128 partitions), (y, c)]
layout; for each feature row y it issues one TensorE matmul
    psum[c, bins] += map_row[x, c].T @ W_y[x, bins]
accumulating bins in PSUM 512-column "generations" (bins sorted by first
active row).  W_y columns are dense per-bin x-profiles scaled by ky[bin, y],
built on the host and streamed per generation.

Sharding: bins are split by (batch, ylo-quantile) into 8 equal-count shards.
The compiled program is shared by all cores; all per-core variation lives in
the input tensors (map slice, W stream).  Cross-core schedule alignment is
per-generation ("virtual steps"), with the map supplied as per-generation row
segments so each core can anchor a generation at its own starting row.
"""
import sys
import time

import numpy as np

sys.path.insert(0, "/opt/trn_rl_repo")

SPATIAL_SCALE = np.float32(0.0625)
POOLED = 7
SAMPLES = 4
TRANS_STD = np.float32(0.1)
B, C, H, W = 2, 128, 128, 128
NCORES = 8
GEN_COLS = 512
DT_MODE = "bf16"  # "f32" (exact, 4-pass PE), "f32r" (1-pass), "bf16" (1-pass, half DMA)

f32 = np.float32


def _np_stream_dt():
    if DT_MODE == "bf16":
        import ml_dtypes
        return ml_dtypes.bfloat16
    return f32
YSENT = 10 ** 6  # sentinel ylo for bins with all-zero weights


# ----------------------------------------------------------------------------
# host planning
# ----------------------------------------------------------------------------

def _bin_params(rois, offset):
    """Exact float32 emulation of the reference coordinate math.

    Returns per-bin (N*49) arrays: batch, dense hat profiles kx/ky [nb, 128]
    (ky has 1/count folded in), y-support [ylo, yhi], zero-bin mask.
    """
    N = rois.shape[0]
    P, S = POOLED, SAMPLES
    rois = rois.astype(f32)
    offset = offset.astype(f32)

    batch_ind = rois[:, 0].astype(np.int32)
    roi_sw = np.round(rois[:, 1]) * SPATIAL_SCALE - f32(0.5)
    roi_sh = np.round(rois[:, 2]) * SPATIAL_SCALE - f32(0.5)
    roi_ew = np.round(rois[:, 3] + f32(1.0)) * SPATIAL_SCALE - f32(0.5)
    roi_eh = np.round(rois[:, 4] + f32(1.0)) * SPATIAL_SCALE - f32(0.5)
    roi_w = np.maximum(roi_ew - roi_sw, f32(0.1))
    roi_h = np.maximum(roi_eh - roi_sh, f32(0.1))
    bin_w = roi_w / f32(P)
    bin_h = roi_h / f32(P)
    sub_w = bin_w / f32(S)
    sub_h = bin_h / f32(S)

    pidx = np.arange(P, dtype=f32)
    trans_x = offset[:, 0] * TRANS_STD          # [N, 7(ph), 7(pw)]
    trans_y = offset[:, 1] * TRANS_STD
    pw = pidx[None, None, :]
    ph = pidx[None, :, None]
    wstart = pw * bin_w[:, None, None] + roi_sw[:, None, None] + trans_x * roi_w[:, None, None]
    hstart = ph * bin_h[:, None, None] + roi_sh[:, None, None] + trans_y * roi_h[:, None, None]

    sidx = np.arange(S, dtype=f32)
    w_s = wstart[..., None] + sidx * sub_w[:, None, None, None]     # [N,7,7,4]
    h_s = hstart[..., None] + sidx * sub_h[:, None, None, None]
    mask_w = (w_s >= f32(-0.5)) & (w_s <= f32(W) - f32(0.5))
    mask_h = (h_s >= f32(-0.5)) & (h_s <= f32(H) - f32(0.5))
    wc = np.clip(w_s, f32(0.0), f32(W - 1))
    hc = np.clip(h_s, f32(0.0), f32(H - 1))

    cnt = (mask_h.sum(-1) * mask_w.sum(-1)).astype(f32)             # [N,7,7]
    inv = np.where(cnt > 0, f32(1.0) / np.maximum(cnt, f32(1.0)), f32(0.0))

    nb = N * P * P
    wc = wc.reshape(nb, S)
    hc = hc.reshape(nb, S)
    mask_w = mask_w.reshape(nb, S)
    mask_h = mask_h.reshape(nb, S)
    inv = inv.reshape(nb)

    xg = np.arange(W, dtype=np.float64)
    kx = np.zeros((nb, W), np.float64)
    ky = np.zeros((nb, H), np.float64)
    for s in range(S):
        kx += mask_w[:, s, None] * np.maximum(0.0, 1.0 - np.abs(wc[:, s, None].astype(np.float64) - xg))
        ky += mask_h[:, s, None] * np.maximum(0.0, 1.0 - np.abs(hc[:, s, None].astype(np.float64) - xg))
    ky *= inv[:, None]
    kx = kx.astype(f32)
    ky = ky.astype(f32)

    ky_nz = ky != 0
    has_y = ky_nz.any(axis=1)
    ylo = np.where(has_y, ky_nz.argmax(axis=1), YSENT).astype(np.int64)
    yhi = np.where(has_y, H - 1 - ky_nz[:, ::-1].argmax(axis=1), -YSENT).astype(np.int64)

    batch = np.repeat(batch_ind, P * P)
    return batch, kx, ky, ylo, yhi, has_y


def _plan(rois, offset):
    batch, kx, ky, ylo, yhi, has_y = _bin_params(rois, offset)

    # shard bins: (batch, ylo-quantile) -> 8 shards with equal-ish counts
    shards = []
    for b in range(B):
        ids = np.where(batch == b)[0]
        # secondary yhi sort shrinks the retirement-prefix blocking window
        ids = ids[np.lexsort((yhi[ids], ylo[ids]))]
        q = NCORES // B
        shards.extend(ids[int(len(ids) * i / q):int(len(ids) * (i + 1) / q)]
                      for i in range(q))
    assert len(shards) == NCORES

    max_count = max(len(s) for s in shards)
    ngens = max(1, -(-max_count // GEN_COLS))
    nslots = ngens * GEN_COLS

    # per-shard absolute row window
    row_start = np.zeros(NCORES, np.int64)
    extents = []
    for ids in shards:
        real = ids[ylo[ids] < YSENT]
        if len(real):
            extents.append((int(ylo[real].min()), int(yhi[real].max())))
        else:
            extents.append((0, 0))
    rstar = max(b_ - a_ + 1 for a_, b_ in extents)
    rstar = min(H, -(-rstar // 8) * 8)
    for ci, (a_, b_) in enumerate(extents):
        row_start[ci] = min(a_, H - rstar)

    # per (core, gen): local anchor t0 and per-step lo/hi profiles
    t0 = np.zeros((NCORES, ngens), np.int64)         # anchor row (local)
    span = np.zeros((NCORES, ngens), np.int64)       # steps used by this core
    profiles = {}                                    # (ci, g) -> (los, his) arrays
    for ci, ids in enumerate(shards):
        rs = int(row_start[ci])
        for g in range(ngens):
            gids = ids[g * GEN_COLS:(g + 1) * GEN_COLS]
            real_mask = ylo[gids] < YSENT
            real = gids[real_mask]
            if len(real) == 0:
                continue
            yl_r = ylo[real] - rs
            yh_r = yhi[real] - rs
            a_, b_ = int(yl_r.min()), int(yh_r.max())
            t0[ci, g] = a_
            span[ci, g] = b_ - a_ + 1
            # profiles over the gen's slot list (real bins are a prefix of the
            # slot list because sentinels sort last)
            yl = ylo[gids] - rs
            yh = yhi[gids] - rs
            los, his = [], []
            for s in range(b_ - a_ + 1):
                t = a_ + s
                his.append(int(np.count_nonzero(yl <= t)))
                live = np.where(yh >= t)[0]
                los.append(int(live[0]) if len(live) else his[-1])
            profiles[(ci, g)] = (los, his)

    # shared schedule: per gen, steps s in [0, S_g); envelope widths
    sched = []          # (g, s, col_lo, col_hi, first, last)
    seg_rows = []       # steps per gen (map segment sizes)
    for g in range(ngens):
        S_g = int(span[:, g].max()) if span[:, g].max() > 0 else 0
        seg_rows.append(S_g)
        if S_g == 0:
            continue
        al = 8 if DT_MODE == "f32r" else 1   # fp32r ISA needs 8-aligned psum cols
        rows = []
        for s in range(S_g):
            cl, ch = YSENT, 0
            for ci in range(NCORES):
                if (ci, g) not in profiles:
                    continue
                los, his = profiles[(ci, g)]
                if s < len(los) and his[s] > los[s]:
                    cl = min(cl, los[s])
                    ch = max(ch, his[s])
            if ch > cl:
                rows.append((s, cl & ~(al - 1), min(GEN_COLS, -(-ch // al) * al)))
        for i, (s, cl, ch) in enumerate(rows):
            sched.append((g, s, cl, ch, i == 0, i == len(rows) - 1))
    wcols = sum(ch - cl for (_, _, cl, ch, _, _) in sched)
    srows = sum(seg_rows)

    # map segment offsets (rows) and W chunk offsets (cols) per gen
    seg_off = np.concatenate([[0], np.cumsum(seg_rows)]).astype(np.int64)
    gen_wc = {}
    off = 0
    for (g, s, cl, ch, first, last) in sched:
        if first:
            gen_wc[g] = off
        off += ch - cl

    meta = dict(rstar=int(rstar), ngens=int(ngens), nslots=int(nslots),
                sched=tuple(sched), wcols=int(wcols), srows=int(srows),
                seg_rows=tuple(seg_rows), seg_off=tuple(int(x) for x in seg_off),
                gen_wc=tuple(sorted(gen_wc.items())))
    return dict(meta=meta, shards=shards, row_start=row_start, t0=t0,
                kx=kx, ky=ky, ylo=ylo, has_y=has_y)


def _build_inputs(plan, data):
    meta = plan["meta"]
    sched, srows, wcols = meta["sched"], meta["srows"], meta["wcols"]
    seg_off = meta["seg_off"]
    kx, ky = plan["kx"], plan["ky"]
    t0 = plan["t0"]
    sdt = _np_stream_dt()
    data_perm = np.ascontiguousarray(data.transpose(0, 3, 2, 1)).astype(sdt)  # [B, W(x), H(y), C]

    in_maps = []
    for ci in range(NCORES):
        ids = plan["shards"][ci]
        b = ci // (NCORES // B)
        rs = int(plan["row_start"][ci])
        # segmented map: for gen g, S_g rows starting at rs + t0[ci, g]
        mp = np.zeros((128, srows, C), sdt)
        for g, S_g in enumerate(meta["seg_rows"]):
            if S_g == 0:
                continue
            y0 = rs + int(t0[ci, g])
            y1 = min(H, y0 + S_g)
            mp[:, seg_off[g]:seg_off[g] + (y1 - y0), :] = data_perm[b, :, y0:y1, :]
        wbuf = np.zeros((128, wcols), sdt)
        wc_off = 0
        for (g, s, cl, ch, first, last) in sched:
            width = ch - cl
            y = rs + int(t0[ci, g]) + s
            if y < H:
                gids = ids[g * GEN_COLS + cl:g * GEN_COLS + ch]
                if len(gids):
                    vals = kx[gids] * ky[gids, y][:, None]      # [ncols_real, 128]
                    wbuf[:, wc_off:wc_off + len(gids)] = vals.T.astype(sdt)
            wc_off += width
        in_maps.append({"mp": np.ascontiguousarray(mp.reshape(128, srows * C)),
                        "w": wbuf})
    return in_maps


# ----------------------------------------------------------------------------
# device program
# ----------------------------------------------------------------------------

def _split_drains(nc, mybir, bass_rust):
    for f_ in nc.m.functions:
        for blk in f_.blocks:
            newlist = []
            for ins in blk.instructions:
                wts = list(ins.sync_info.on_wait) if ins.sync_info else []
                if len(wts) > 1 and type(ins).__name__ == "InstDrain":
                    for j, wx in enumerate(wts[1:]):
                        nop = mybir.InstNoOp(name=f"splitw_{id(ins)}_{j}", ins=[], outs=[])
                        nop.engine = ins.engine
                        nop.sync_info = bass_rust.SyncInfo(on_wait=[wx], on_update=[])
                        newlist.append(nop)
                    ins.sync_info.on_wait = wts[:1]
                newlist.append(ins)
            blk.instructions = newlist


def _build_program(meta, rep=1):
    import concourse.bacc as bacc
    import concourse.mybir as mybir
    import bass_rust
    from concourse.tile import TileContext

    ngens, nslots = meta["ngens"], meta["nslots"]
    sched, wcols, srows = meta["sched"], meta["wcols"], meta["srows"]
    seg_off = meta["seg_off"]
    gen_wc = dict(meta["gen_wc"])
    dt = {"f32r": mybir.dt.float32r, "bf16": mybir.dt.bfloat16}.get(DT_MODE, mybir.dt.float32)
    odt = mybir.dt.bfloat16 if DT_MODE == "bf16" else mybir.dt.float32

    # per-gen W chunk extents
    gen_wend = {}
    off = 0
    for (g, s, cl, ch, first, last) in sched:
        off += ch - cl
        gen_wend[g] = off

    nc = bacc.Bacc()
    mp = nc.declare_dram_parameter("mp", [128, srows * C], dt, isOutput=False)
    w = nc.declare_dram_parameter("w", [128, max(wcols, 8)], dt, isOutput=False)
    o = nc.declare_dram_parameter("o", [128, nslots], odt, isOutput=True)

    with TileContext(nc) as tc:
        with (
            tc.tile_pool(name="const", bufs=1) as constp,
            tc.tile_pool(name="mapp", bufs=2) as mpool,
            tc.tile_pool(name="wp", bufs=3) as wpool,
            tc.tile_pool(name="ps", bufs=2, space="PSUM") as pspool,
        ):
            stage = constp.tile([128, nslots], odt)
            wmax = max((gen_wend[g] - gen_wc[g] for g in gen_wc), default=8)
            for _rep in range(rep):
                map_t = mpool.tile([128, srows * C], dt, tag="map")
                nload = min(16, srows)
                bounds = [int(srows * i / nload) for i in range(nload + 1)]
                for i in range(nload):
                    r0, r1 = bounds[i], bounds[i + 1]
                    if r1 > r0:
                        nc.sync.dma_start(out=map_t[:, r0 * C:r1 * C], in_=mp[:, r0 * C:r1 * C])
                ps = None
                w_t = None
                cur_g = -1
                wc_off = 0
                for (g, s, cl, ch, first, last) in sched:
                    width = ch - cl
                    if g != cur_g:
                        ps = pspool.tile([128, GEN_COLS], mybir.dt.float32, tag="ps")
                        w_t = wpool.tile([128, wmax], dt, tag="wt")
                        # balance the two HWDGE rings: W mostly on ACT, first
                        # gen on SP (SP also carries map+out)
                        weng = nc.sync if g == 0 else nc.scalar
                        weng.dma_start(out=w_t[:, :gen_wend[g] - gen_wc[g]],
                                       in_=w[:, gen_wc[g]:gen_wend[g]])
                        cur_g = g
                        wc_off = 0
                    row = map_t[:, (seg_off[g] + s) * C:(seg_off[g] + s + 1) * C]
                    nc.tensor.matmul(ps[:, cl:ch], row, w_t[:, wc_off:wc_off + width],
                                     start=first, stop=last)
                    wc_off += width
                    if last:
                        sl = slice(g * GEN_COLS, (g + 1) * GEN_COLS)
                        nc.vector.tensor_copy(stage[:, sl], ps[:])
                        # drain each generation's output immediately so the
                        # store overlaps later generations' compute
                        nc.sync.dma_start(out=o[:, sl], in_=stage[:, sl])

    _split_drains(nc, mybir, bass_rust)
    nc.finalize()
    return nc


_prog_cache = {}


def _get_program(meta, rep=1):
    key = (meta["sched"], meta["srows"], meta["nslots"], rep, DT_MODE)
    if key not in _prog_cache:
        _prog_cache[key] = _build_program(meta, rep=rep)
    return _prog_cache[key]


def _run(nc, in_maps):
    from concourse.bass_utils import run_bass_kernel_spmd
    last_err = None
    for _attempt in range(3):
        try:
            res = run_bass_kernel_spmd(nc, in_maps, list(range(NCORES)))
            return res.results
        except Exception as e:  # transient device wedge -> retry
            last_err = e
            time.sleep(2.0)
    raise last_err


# ----------------------------------------------------------------------------
# public entry
# ----------------------------------------------------------------------------

def kernel(data, rois, offset):
    data = np.asarray(data, f32)
    rois = np.asarray(rois, f32)
    offset = np.asarray(offset, f32)
    N = rois.shape[0]

    plan = _plan(rois, offset)
    if len(plan["meta"]["sched"]) == 0:   # every bin fully masked
        return np.zeros((N, C, POOLED, POOLED), f32)
    in_maps = _build_inputs(plan, data)
    nc = _get_program(plan["meta"])
    results = _run(nc, in_maps)

    flat = np.zeros((N * POOLED * POOLED, C), f32)   # [bin, c]
    for ci in range(NCORES):
        ids = plan["shards"][ci]
        if len(ids) == 0:
            continue
        sb = np.asarray(results[ci]["o"]).astype(f32)  # [128, nslots]
        flat[ids] = sb[:, :len(ids)].T
    flat[~plan["has_y"]] = 0.0   # degenerate bins never touched on device
    out = flat.reshape(N, POOLED, POOLED, C).transpose(0, 3, 1, 2)
    return np.ascontiguousarray(out)



# revision 14
# speedup vs baseline: 2.4723x; 1.5962x over previous
"""Deformable PSROI pooling (group_size=1, num_classes=1) on 8 trn2 NeuronCores.

Strategy ("x-strip map sweep"):
  out[n, c, ph, pw] = sum_{y,x} KY[bin, y] * KX[bin, x] * data[b, c, y, x]
where KX/KY are per-bin bilinear "hat" weight profiles (sums over the 4x4
sample grid, with sample masks and 1/count folded in).  KX support is <= 5
consecutive x columns and KY support <= 5 consecutive y rows.

Sharding: bins are sharded by (batch, x-quantile).  Each core holds only its
x-strip of the feature map ([KX ~ 40 partitions, all 128 rows, C]) in SBUF,
loaded once per rep (union map, no per-generation segment duplication).  For
each feature row y it issues one TensorE matmul
    psum[c, cols] += strip_row[x, c].T @ W_y[x, cols]
with contraction K = KX (not 128), so the streamed W is ~3x smaller.

Column layout ("row-aligned slots"): for each absolute feature row r the
schedule reserves wmax[r] = max_core #bins-with-ylo==r columns; every core
places its row-r bins at the shared slot base.  Each bin is treated as active
for exactly PAD=5 sweep rows [ylo, ylo+5) (ky is zero outside its true
support), so the active columns at sweep row y are exactly the slots of rows
y-4..y: a contiguous, monotone sliding window shared by all cores with no
per-core anchoring.  Generations = consecutive row groups holding <= 512
slots (one PSUM bank); a generation's sweep extends PAD-1 rows past its last
row so every bin completes within its own generation.
"""
import sys
import time

import numpy as np

sys.path.insert(0, "/opt/trn_rl_repo")

SPATIAL_SCALE = np.float32(0.0625)
POOLED = 7
SAMPLES = 4
TRANS_STD = np.float32(0.1)
B, C, H, W = 2, 128, 128, 128
NCORES = 8
GEN_COLS = 512
PAD = 5          # max bilinear support height/width in feature rows
DT_MODE = "bf16"

f32 = np.float32


def _np_stream_dt():
    if DT_MODE == "bf16":
        import ml_dtypes
        return ml_dtypes.bfloat16
    return f32


# ----------------------------------------------------------------------------
# host planning
# ----------------------------------------------------------------------------

def _bin_params(rois, offset):
    """Exact float32 emulation of the reference coordinate math.

    Returns per-bin (N*49) arrays: batch, dense hat profiles kx/ky [nb, 128]
    (ky has 1/count folded in), y-support [ylo, yhi], x-support [xlo, xhi],
    validity mask.
    """
    N = rois.shape[0]
    P, S = POOLED, SAMPLES
    rois = rois.astype(f32)
    offset = offset.astype(f32)

    batch_ind = rois[:, 0].astype(np.int32)
    roi_sw = np.round(rois[:, 1]) * SPATIAL_SCALE - f32(0.5)
    roi_sh = np.round(rois[:, 2]) * SPATIAL_SCALE - f32(0.5)
    roi_ew = np.round(rois[:, 3] + f32(1.0)) * SPATIAL_SCALE - f32(0.5)
    roi_eh = np.round(rois[:, 4] + f32(1.0)) * SPATIAL_SCALE - f32(0.5)
    roi_w = np.maximum(roi_ew - roi_sw, f32(0.1))
    roi_h = np.maximum(roi_eh - roi_sh, f32(0.1))
    bin_w = roi_w / f32(P)
    bin_h = roi_h / f32(P)
    sub_w = bin_w / f32(S)
    sub_h = bin_h / f32(S)

    pidx = np.arange(P, dtype=f32)
    trans_x = offset[:, 0] * TRANS_STD          # [N, 7(ph), 7(pw)]
    trans_y = offset[:, 1] * TRANS_STD
    pw = pidx[None, None, :]
    ph = pidx[None, :, None]
    wstart = pw * bin_w[:, None, None] + roi_sw[:, None, None] + trans_x * roi_w[:, None, None]
    hstart = ph * bin_h[:, None, None] + roi_sh[:, None, None] + trans_y * roi_h[:, None, None]

    sidx = np.arange(S, dtype=f32)
    w_s = wstart[..., None] + sidx * sub_w[:, None, None, None]     # [N,7,7,4]
    h_s = hstart[..., None] + sidx * sub_h[:, None, None, None]
    mask_w = (w_s >= f32(-0.5)) & (w_s <= f32(W) - f32(0.5))
    mask_h = (h_s >= f32(-0.5)) & (h_s <= f32(H) - f32(0.5))
    wc = np.clip(w_s, f32(0.0), f32(W - 1))
    hc = np.clip(h_s, f32(0.0), f32(H - 1))

    cnt = (mask_h.sum(-1) * mask_w.sum(-1)).astype(f32)             # [N,7,7]
    inv = np.where(cnt > 0, f32(1.0) / np.maximum(cnt, f32(1.0)), f32(0.0))

    nb = N * P * P
    wc = wc.reshape(nb, S)
    hc = hc.reshape(nb, S)
    mask_w = mask_w.reshape(nb, S)
    mask_h = mask_h.reshape(nb, S)
    inv = inv.reshape(nb)

    xg = np.arange(W, dtype=np.float64)
    kx = np.zeros((nb, W), np.float64)
    ky = np.zeros((nb, H), np.float64)
    for s in range(S):
        kx += mask_w[:, s, None] * np.maximum(0.0, 1.0 - np.abs(wc[:, s, None].astype(np.float64) - xg))
        ky += mask_h[:, s, None] * np.maximum(0.0, 1.0 - np.abs(hc[:, s, None].astype(np.float64) - xg))
    ky *= inv[:, None]
    kx = kx.astype(f32)
    ky = ky.astype(f32)

    ky_nz = ky != 0
    has_y = ky_nz.any(axis=1)
    ylo = np.where(has_y, ky_nz.argmax(axis=1), 0).astype(np.int64)
    yhi = np.where(has_y, H - 1 - ky_nz[:, ::-1].argmax(axis=1), -1).astype(np.int64)
    kx_nz = kx != 0
    has_x = kx_nz.any(axis=1)
    xlo = np.where(has_x, kx_nz.argmax(axis=1), 0).astype(np.int64)
    xhi = np.where(has_x, W - 1 - kx_nz[:, ::-1].argmax(axis=1), -1).astype(np.int64)

    batch = np.repeat(batch_ind, P * P)
    real = has_y & has_x
    return batch, kx, ky, ylo, yhi, xlo, xhi, real


def _plan(rois, offset):
    batch, kx, ky, ylo, yhi, xlo, xhi, real = _bin_params(rois, offset)

    # shard real bins: (batch, x-quantile) -> 8 equal-count strips
    shards = []
    for b in range(B):
        ids = np.where((batch == b) & real)[0]
        ids = ids[np.lexsort((xhi[ids], xlo[ids]))]
        q = NCORES // B
        shards.extend(ids[int(len(ids) * i / q):int(len(ids) * (i + 1) / q)]
                      for i in range(q))
    assert len(shards) == NCORES

    # shared strip width KX; per-core strip origin x0
    KX = max((int(xhi[ids].max() - xlo[ids].min() + 1) if len(ids) else 1)
             for ids in shards)
    KX = min(W, -(-KX // 4) * 4)   # round up for tidy DMA
    x0 = np.zeros(NCORES, np.int64)
    for ci, ids in enumerate(shards):
        if len(ids):
            x0[ci] = min(int(xlo[ids].min()), W - KX)

    # rebalance per (batch, ylo-row): move strip-overlap bins to the least
    # loaded feasible strip -- reduces sum_r max_core count (slots & W cols)
    sel = np.empty(batch.shape[0], np.int64)
    for ci, ids in enumerate(shards):
        sel[ids] = ci
    for b in range(B):
        cores = list(range(b * (NCORES // B), (b + 1) * (NCORES // B)))
        ids_b = np.concatenate([shards[ci] for ci in cores])
        for r in range(H):
            rm = ids_b[ylo[ids_b] == r]
            if len(rm) == 0:
                continue
            cnt = {ci: int(np.count_nonzero(sel[rm] == ci)) for ci in cores}
            for _ in range(64):
                hi_c = max(cores, key=lambda c: cnt[c])
                moved = False
                for i in rm:
                    if sel[i] != hi_c:
                        continue
                    for lo_c in sorted(cores, key=lambda c: cnt[c]):
                        if cnt[lo_c] >= cnt[hi_c] - 1:
                            break
                        if x0[lo_c] <= xlo[i] and xhi[i] < x0[lo_c] + KX:
                            sel[i] = lo_c
                            cnt[hi_c] -= 1
                            cnt[lo_c] += 1
                            moved = True
                            break
                    if moved:
                        break
                if not moved:
                    break
        for ci in cores:
            shards[ci] = ids_b[sel[ids_b] == ci]

    # per-core bins sorted by (ylo, yhi); per-row id lists
    percore = []
    for ids in shards:
        o = np.lexsort((yhi[ids], ylo[ids]))
        percore.append(ids[o])

    # row-aligned slots: wmax[r] = max over cores of #bins with ylo == r
    cnt_cr = np.zeros((NCORES, H), np.int64)
    for ci in range(NCORES):
        r, c = np.unique(ylo[percore[ci]], return_counts=True)
        cnt_cr[ci, r] = c
    wmax = cnt_cr.max(axis=0)                       # [H]
    base = np.concatenate([[0], np.cumsum(wmax)])   # [H+1]

    # generations: consecutive rows with <= GEN_COLS slots
    bands = [0]
    for r in range(H):
        if base[r + 1] - base[bands[-1]] > GEN_COLS:
            bands.append(r)
    bands.append(H)
    ngens = len(bands) - 1
    nslots = ngens * GEN_COLS

    # schedule rows: per gen, sweep y in [Ra, min(Rb-1+PAD, H));
    # active slots at y = rows max(Ra, y-PAD+1) .. min(y, Rb-1)
    sched = []   # (g, y, cl, ch, first, last)
    for g in range(ngens):
        Ra, Rb = bands[g], bands[g + 1]
        if base[Rb] == base[Ra]:
            continue
        rows = []
        for y in range(Ra, min(Rb - 1 + PAD, H)):
            cl = int(base[max(Ra, y - PAD + 1)] - base[Ra])
            ch = int(min(base[y + 1], base[Rb]) - base[Ra])
            if ch > cl:
                rows.append((y, cl, ch))
        for i, (y, cl, ch) in enumerate(rows):
            sched.append((g, y, cl, ch, i == 0, i == len(rows) - 1))
    wcols = sum(ch - cl for (_, _, cl, ch, _, _) in sched)

    # per-gen W chunk offsets
    gen_wc = {}
    off = 0
    for (g, y, cl, ch, first, last) in sched:
        if first:
            gen_wc[g] = off
        off += ch - cl

    meta = dict(KX=int(KX), ngens=int(ngens), nslots=int(nslots),
                sched=tuple(sched), wcols=int(wcols),
                bands=tuple(bands), gen_wc=tuple(sorted(gen_wc.items())),
                # kept for test.py's stat line
                rstar=int(H), srows=int(H))
    return dict(meta=meta, shards=shards, percore=percore, x0=x0,
                base=base, cnt_cr=cnt_cr, kx=kx, ky=ky, ylo=ylo, real=real)


def _build_inputs(plan, data):
    meta = plan["meta"]
    KX, sched, wcols = meta["KX"], meta["sched"], meta["wcols"]
    bands = meta["bands"]
    base = plan["base"]
    kx, ky, ylo = plan["kx"], plan["ky"], plan["ylo"]
    sdt = _np_stream_dt()
    data_perm = np.ascontiguousarray(data.transpose(0, 3, 2, 1)).astype(sdt)  # [B, W(x), H(y), C]

    in_maps = []
    for ci in range(NCORES):
        ids = plan["percore"][ci]
        b = ci // (NCORES // B)
        xs = int(plan["x0"][ci])
        mp = np.ascontiguousarray(data_perm[b, xs:xs + KX].reshape(KX, H * C))

        # per-row bin ids and slot columns (within-gen)
        yl = ylo[ids]
        rowptr = np.concatenate([[0], np.cumsum(np.bincount(yl, minlength=H))])
        # column of each bin inside its gen
        colof = np.empty(len(ids), np.int64)
        for g in range(len(bands) - 1):
            Ra, Rb = bands[g], bands[g + 1]
            for r in range(Ra, Rb):
                i0, i1 = rowptr[r], rowptr[r + 1]
                c0 = base[r] - base[Ra]
                colof[i0:i1] = np.arange(i1 - i0) + c0

        wbuf = np.zeros((KX, max(wcols, 8)), sdt)
        wc_off = 0
        for (g, y, cl, ch, first, last) in sched:
            width = ch - cl
            Ra, Rb = bands[g], bands[g + 1]
            rlo, rhi = max(Ra, y - PAD + 1), min(y, Rb - 1)
            i0, i1 = rowptr[rlo], rowptr[rhi + 1]
            if i1 > i0:
                gids = ids[i0:i1]
                vals = kx[gids, xs:xs + KX] * ky[gids, y][:, None]   # [n, KX]
                wbuf[:, wc_off + (colof[i0:i1] - cl)] = vals.T.astype(sdt)
            wc_off += width
        in_maps.append({"mp": mp, "w": wbuf})
    return in_maps


# ----------------------------------------------------------------------------
# device program
# ----------------------------------------------------------------------------

def _split_drains(nc, mybir, bass_rust):
    for f_ in nc.m.functions:
        for blk in f_.blocks:
            newlist = []
            for ins in blk.instructions:
                wts = list(ins.sync_info.on_wait) if ins.sync_info else []
                if len(wts) > 1 and type(ins).__name__ == "InstDrain":
                    for j, wx in enumerate(wts[1:]):
                        nop = mybir.InstNoOp(name=f"splitw_{id(ins)}_{j}", ins=[], outs=[])
                        nop.engine = ins.engine
                        nop.sync_info = bass_rust.SyncInfo(on_wait=[wx], on_update=[])
                        newlist.append(nop)
                    ins.sync_info.on_wait = wts[:1]
                newlist.append(ins)
            blk.instructions = newlist


def _build_program(meta, rep=1):
    import concourse.bacc as bacc
    import concourse.mybir as mybir
    import bass_rust
    from concourse.tile import TileContext

    KX, ngens, nslots = meta["KX"], meta["ngens"], meta["nslots"]
    sched, wcols = meta["sched"], meta["wcols"]
    gen_wc = dict(meta["gen_wc"])
    dt = {"f32r": mybir.dt.float32r, "bf16": mybir.dt.bfloat16}.get(DT_MODE, mybir.dt.float32)
    odt = mybir.dt.bfloat16 if DT_MODE == "bf16" else mybir.dt.float32

    gen_wend = {}
    gen_used = {}
    off = 0
    for (g, y, cl, ch, first, last) in sched:
        off += ch - cl
        gen_wend[g] = off
        gen_used[g] = max(gen_used.get(g, 0), ch)

    nc = bacc.Bacc()
    mp = nc.declare_dram_parameter("mp", [KX, H * C], dt, isOutput=False)
    w = nc.declare_dram_parameter("w", [KX, max(wcols, 8)], dt, isOutput=False)
    o = nc.declare_dram_parameter("o", [128, nslots], odt, isOutput=True)

    with TileContext(nc) as tc:
        with (
            tc.tile_pool(name="const", bufs=1) as constp,
            tc.tile_pool(name="mapp", bufs=2) as mpool,
            tc.tile_pool(name="wp", bufs=3) as wpool,
            tc.tile_pool(name="ps", bufs=2, space="PSUM") as pspool,
        ):
            stage = constp.tile([128, nslots], odt)
            wmax_chunk = max((gen_wend[g] - gen_wc[g] for g in gen_wc), default=8)
            for _rep in range(rep):
                map_t = mpool.tile([KX, H * C], dt, tag="map")
                nload = 16
                bounds = [H * i // nload for i in range(nload + 1)]
                for i in range(nload):
                    r0, r1 = bounds[i], bounds[i + 1]
                    if r1 > r0:
                        nc.sync.dma_start(out=map_t[:, r0 * C:r1 * C], in_=mp[:, r0 * C:r1 * C])
                ps = None
                w_t = None
                cur_g = -1
                wc_off = 0
                for (g, y, cl, ch, first, last) in sched:
                    width = ch - cl
                    if g != cur_g:
                        ps = pspool.tile([128, GEN_COLS], mybir.dt.float32, tag="ps")
                        w_t = wpool.tile([KX, wmax_chunk], dt, tag="wt")
                        # W stream on the ACT HWDGE ring; map+out on SP
                        nc.scalar.dma_start(out=w_t[:, :gen_wend[g] - gen_wc[g]],
                                            in_=w[:, gen_wc[g]:gen_wend[g]])
                        cur_g = g
                        wc_off = 0
                    row = map_t[:, y * C:(y + 1) * C]
                    nc.tensor.matmul(ps[:, cl:ch], row, w_t[:, wc_off:wc_off + width],
                                     start=first, stop=last)
                    wc_off += width
                    if last:
                        used = gen_used[g]
                        sl = slice(g * GEN_COLS, g * GEN_COLS + used)
                        nc.vector.tensor_copy(stage[:, sl], ps[:, :used])
                        # drain each generation's output immediately so the
                        # store overlaps later generations' compute
                        nc.sync.dma_start(out=o[:, sl], in_=stage[:, sl])

    _split_drains(nc, mybir, bass_rust)
    nc.finalize()
    return nc


_prog_cache = {}


def _get_program(meta, rep=1):
    key = (meta["sched"], meta["KX"], meta["nslots"], rep, DT_MODE)
    if key not in _prog_cache:
        _prog_cache[key] = _build_program(meta, rep=rep)
    return _prog_cache[key]


def _run(nc, in_maps):
    from concourse.bass_utils import run_bass_kernel_spmd
    last_err = None
    for _attempt in range(3):
        try:
            res = run_bass_kernel_spmd(nc, in_maps, list(range(NCORES)))
            return res.results
        except Exception as e:  # transient device wedge -> retry
            last_err = e
            time.sleep(2.0)
    raise last_err


# ----------------------------------------------------------------------------
# public entry
# ----------------------------------------------------------------------------

def kernel(data, rois, offset):
    data = np.asarray(data, f32)
    rois = np.asarray(rois, f32)
    offset = np.asarray(offset, f32)
    N = rois.shape[0]

    plan = _plan(rois, offset)
    meta = plan["meta"]
    if len(meta["sched"]) == 0:   # every bin fully masked
        return np.zeros((N, C, POOLED, POOLED), f32)
    in_maps = _build_inputs(plan, data)
    nc = _get_program(meta)
    results = _run(nc, in_maps)

    bands = meta["bands"]
    base = plan["base"]
    flat = np.zeros((N * POOLED * POOLED, C), f32)   # [bin, c]
    for ci in range(NCORES):
        ids = plan["percore"][ci]
        if len(ids) == 0:
            continue
        yl = plan["ylo"][ids]
        rowptr = np.concatenate([[0], np.cumsum(np.bincount(yl, minlength=H))])
        colof = np.empty(len(ids), np.int64)
        for g in range(len(bands) - 1):
            Ra, Rb = bands[g], bands[g + 1]
            for r in range(Ra, Rb):
                i0, i1 = rowptr[r], rowptr[r + 1]
                colof[i0:i1] = g * GEN_COLS + (base[r] - base[Ra]) + np.arange(i1 - i0)
        sb = np.asarray(results[ci]["o"]).astype(f32)  # [128, nslots]
        flat[ids] = sb[:, colof].T
    flat[~plan["real"]] = 0.0
    out = flat.reshape(N, POOLED, POOLED, C).transpose(0, 3, 1, 2)
    return np.ascontiguousarray(out)
